# revision 4
# baseline (speedup 1.0000x reference)
"""Trainium2 (Bass/Tile) multi-head attention kernel, v2.

Problem: nn_MultiHeadAttention (B=4, T=2048, C=1024, H=16 heads, D=64),
fp32, causal, torch-Linear-style projections (y = x @ W.T + b).

Sharding (8 NeuronCores): data-parallel over B (4) x tensor-parallel over
head-groups (2 groups of 8 heads). Core c handles batch c//2, head group
c%2.

v2 changes vs baseline:
  - Host pre-arranges every input into the on-chip [128, ...] layout so
    each DMA run is >=4KB contiguous per partition (startup was 25.8us of
    1KB-fragment gathers).
  - x^T is loaded once and stays resident in SBUF (baseline reloaded each
    512-token slice from DRAM 5x).
  - kt iterations are processed in PAIRS: the four K=64 S^T matmuls of two
    k-tiles chain in 64x128 array mode, paying the 128<->64 tiling-mode
    drain (~107ns) once per pair instead of once per kt.
  - PV runs one kt-pair behind S, so the ACT exp latency never stalls the
    PE queue.
  - chunk 3 processes q-tiles in descending order so the final epilogue +
    output projection tail is the cheapest q-tile.
"""

import numpy as np
import ml_dtypes

import bass_rust
import concourse.bass as bass
import concourse.mybir as mybir
import concourse.tile as tile
from concourse.bass_utils import run_bass_kernel_spmd
from concourse.vector_clock import ScopedClock

BF16 = ml_dtypes.bfloat16

B, T, C, H, D = 4, 2048, 1024, 16, 64
G = C // 2          # features per head group (8 heads x 64)
N_CORES = 8
MASK_NEG = -800.0   # pre-scale; exp(0.125 * (s - 800)) == 0 for |s| < 30

# ---------------------------------------------------------------------------
# walrus sync-wait splitting (this build allows ~1 wait per instruction)
# ---------------------------------------------------------------------------


def _patched_drain_and_barrier(self, tick_clock, wait_clock):
    nc = self.nc
    drain_inst = nc.sync.drain()
    wait_clock.add_sem_waits(
        drain_inst.ins, ScopedClock({None: tick_clock.global_clock})
    )
    si = drain_inst.ins.sync_info
    waits = list(si.on_wait) if si is not None else []
    if waits:
        drain_inst.ins.sync_info = bass_rust.SyncInfo(
            on_wait=[], on_update=list(si.on_update)
        )
        assert self.sems is not None
        by_name = {h.name: h for h in self.sems.allocated().values()}
        for w in waits:
            assert w.wait_mode == "sem-ge-imm", w
            nc.sync.wait_ge(by_name[w.ant_name], w.wait_value)

    nc.all_engine_barrier()
    assert self.sems is not None
    popped = nc._tile_sem_poison_stack.pop()
    assert popped is self._sem_poison
    nc.clear_and_free_semaphores(list(self.sems.allocated().values()))
    nc.all_engine_barrier()


tile.TileContext._drain_and_barrier = _patched_drain_and_barrier


def _split_excess_waits(nc, max_waits=1):
    n = 0
    for fn in nc.m.functions:
        for blk in fn.blocks:
            new_insts = []
            for inst in blk.instructions:
                si = inst.sync_info
                waits = list(si.on_wait) if si is not None else []
                if len(waits) > max_waits:
                    for w in waits[:-max_waits]:
                        ev = mybir.InstEventSemaphore(
                            name=f"I-wsplit-{n}", ins=[], outs=[]
                        )
                        n += 1
                        ev.engine = inst.engine
                        ev.sync_info = bass_rust.SyncInfo(
                            on_wait=[w], on_update=[]
                        )
                        new_insts.append(ev)
                    inst.sync_info = bass_rust.SyncInfo(
                        on_wait=waits[-max_waits:], on_update=list(si.on_update)
                    )
                new_insts.append(inst)
            blk.instructions = new_insts


# ---------------------------------------------------------------------------
# Kernel builder (per-core program; same program on all 8 cores)
# ---------------------------------------------------------------------------

def build_nc(t=T, split_waits=True):
    f32 = mybir.dt.float32
    bf16 = mybir.dt.bfloat16
    Exp = mybir.ActivationFunctionType.Exp

    assert t % 512 == 0
    TS = t // 512            # 512-token slices (q-tiles)
    TK = t // 128            # 128-token k-tiles

    nc = bass.Bass()
    # all inputs already in device layout; per-partition-contiguous rows
    x_d = nc.dram_tensor("xt", [t // 512, 128, 8, 512], bf16,
                     kind="ExternalInput")
    wq_d = nc.dram_tensor("wqt", [128, 8, G], bf16, kind="ExternalInput")
    wk_d = nc.dram_tensor("wkt", [128, 8, G], bf16, kind="ExternalInput")
    wv_d = nc.dram_tensor("wvt", [128, 8, G], bf16, kind="ExternalInput")
    wot_d = nc.dram_tensor("wot", [128, 4, C], bf16, kind="ExternalInput")
    bqk_d = nc.dram_tensor("bqk", [128, 8], f32, kind="ExternalInput")
    bv_d = nc.dram_tensor("bv", [G], f32, kind="ExternalInput")
    mask_d = nc.dram_tensor("mask", [128, 512], f32, kind="ExternalInput")
    out_d = nc.dram_tensor("out", [t, C], f32, kind="ExternalOutput")
    rsc_d = nc.dram_tensor("rscratch", [16, 512], bf16, kind="ExternalOutput")

    with tile.TileContext(nc) as tc:
        with (
            tc.tile_pool(name="big", bufs=1) as big,
            tc.tile_pool(name="weights", bufs=1) as wpool,
            tc.tile_pool(name="pt", bufs=6) as ptpool,
            tc.tile_pool(name="small", bufs=4) as small,
            tc.tile_pool(name="psA", bufs=2, space="PSUM") as psA,
            tc.tile_pool(name="psS", bufs=2, space="PSUM") as psS,
            tc.tile_pool(name="psPV", bufs=2, space="PSUM") as psPV,
        ):
            # ---- persistent SBUF tensors ----
            x_sb = big.tile([128, t // 512, 8, 512], bf16, tag="x")  # x^T resident
            qt_sb = big.tile([128, 4, t], bf16, tag="qt")      # Q^T
            kt_sb = big.tile([128, 4, t], bf16, tag="kt")      # K^T
            vaug = big.tile([128, TK, 8 * 65], bf16, tag="va")  # V + ones col
            ot_sb = big.tile([128, 4, t], bf16, tag="ot")      # O^T normalized

            w_sb = {}
            for name in ("v", "q", "k"):
                w_sb[name] = wpool.tile(
                    [128, 8, G], bf16, tag=f"w{name}", name=f"w{name}")
            bv_sb = wpool.tile([128, G], f32, tag="bv")
            mask_sb = wpool.tile([128, 512], f32, tag="mask")
            bqk_sb = wpool.tile([128, 8], f32, tag="bqk")
            lnr_g = wpool.tile([33, 512], f32, tag="lnrg")
            wot_sb = wpool.tile([128, 4, C], bf16, tag="wot")

            # ---- startup DMAs, priority order, ~256KB apiece ----
            # wv + x slice ts0 first (V projection starts the kernel), then
            # wq/wk (QK proj), then the rest of x, then late-need tensors.
            def dma_w(dst, src, i):
                nc.sync.dma_start(out=dst[:, 2 * i:2 * i + 2, :],
                                  in_=src[:, 2 * i:2 * i + 2, :])

            # small late-use-critical tensors first (bias adds gate all
            # attention), then the first V inputs, then the rest.
            nc.sync.dma_start(out=bqk_sb, in_=bqk_d[:, :])
            nc.sync.dma_start(
                out=bv_sb, in_=bv_d[:].unsqueeze(0).to_broadcast((128, G))
            )
            nc.sync.dma_start(out=mask_sb, in_=mask_d[:, :])
            for i in range(4):
                nc.sync.dma_start(out=x_sb[:, 0, 2 * i:2 * i + 2, :],
                                  in_=x_d[0, :, 2 * i:2 * i + 2, :])
                dma_w(w_sb["v"], wv_d, i)


            nc.vector.memset(lnr_g, 1.0)
            va_ones = vaug[:].rearrange("p k (h x) -> p k h x", x=65)[:, :, :, 64]
            nc.vector.memset(va_ones, 1.0)

            # ---- projection groups (filler units) ----
            def v_half(ts, half):
                """V projection for 2 token sub-tiles (16 matmuls)."""
                for tsub in (2 * half, 2 * half + 1):
                    kt_idx = ts * 4 + tsub
                    ps = psA.tile([128, 512], f32, tag="mm", name="psv")
                    for cc in range(8):
                        nc.tensor.matmul(
                            ps,
                            lhsT=x_sb[:, ts, cc,
                                      tsub * 128:(tsub + 1) * 128],
                            rhs=w_sb["v"][:, cc, :],
                            start=(cc == 0),
                            stop=(cc == 7),
                        )
                    dst = vaug[:, kt_idx, :].rearrange(
                        "p (h x) -> p h x", x=65)[:, :, 0:64]
                    nc.vector.tensor_add(
                        out=dst,
                        in0=ps.rearrange("p (h d) -> p h d", d=64),
                        in1=bv_sb.rearrange("p (h d) -> p h d", d=64),
                    )

            def qk_half(c, ts, name):
                """Q^T or K^T projection tile [feat 128, tok 512] (8 mm)."""
                tsl = slice(ts * 512, (ts + 1) * 512)
                dst, bcol = (qt_sb, 0) if name == "q" else (kt_sb, 4)
                ps = psA.tile([128, 512], f32, tag="mm", name="psqk")
                for cc in range(8):
                    nc.tensor.matmul(
                        ps,
                        lhsT=w_sb[name][:, cc, c * 128:(c + 1) * 128],
                        rhs=x_sb[:, ts, cc, :],
                        start=(cc == 0),
                        stop=(cc == 7),
                    )
                with nc.allow_low_precision(
                    reason="Q^T/K^T stored bf16 for the PE"
                ):
                    nc.vector.tensor_scalar_add(
                        out=dst[:, c, tsl],
                        in0=ps,
                        scalar1=bqk_sb[:, bcol + c:bcol + c + 1],
                    )

            def outproj_group(tt, of):
                """Partial output projection [tok 128, outfeat 512]."""
                ps = psA.tile([128, 512], f32, tag="mm", name="psop")
                for fc in range(4):
                    nc.tensor.matmul(
                        ps,
                        lhsT=ot_sb[:, fc, tt * 128:(tt + 1) * 128],
                        rhs=wot_sb[:, fc, of * 512:(of + 1) * 512],
                        start=(fc == 0),
                        stop=(fc == 3),
                    )
                o_out = small.tile([128, 512], f32, tag="oout")
                nc.vector.tensor_copy(out=o_out, in_=ps)
                nc.sync.dma_start(
                    out=out_d[tt * 128:(tt + 1) * 128,
                              of * 512:(of + 1) * 512],
                    in_=o_out,
                )

            # ---- softmax epilogue (DMA-bounce partition broadcast) ----
            norm_state = {"nf": 0}

            def stage_epilogue(c, qt, pv):
                for hp in range(2):
                    nc.scalar.activation(
                        lnr_g[hp * 32:hp * 32 + 1, :], pv[hp][64:65, :],
                        func=mybir.ActivationFunctionType.Ln,
                    )
                r33 = small.tile([33, 512], bf16, tag="r33")
                nc.scalar.activation(r33, lnr_g, func=Exp, scale=-1.0)
                for hp in range(2):
                    po = hp * 64
                    osl = ot_sb[po:po + 64, c, qt * 512:(qt + 1) * 512]
                    with nc.allow_low_precision(
                        reason="O^T staged bf16; normalized in place"
                    ):
                        nc.vector.tensor_copy(out=osl, in_=pv[hp][0:64, :])
                    slot = norm_state["nf"] % 16
                    norm_state["nf"] += 1
                    nc.sync.dma_start(
                        out=rsc_d[slot:slot + 1, :],
                        in_=r33[hp * 32:hp * 32 + 1, :])
                    bcast = small.tile([128, 512], bf16, tag="bcast")
                    nc.sync.dma_start(
                        out=bcast[po:po + 64, :],
                        in_=rsc_d[slot, :].unsqueeze(0).to_broadcast(
                            (64, 512)),
                    )
                    nc.vector.tensor_mul(
                        out=osl, in0=osl, in1=bcast[po:po + 64, :])

            # ---- attention ----
            def s_matmuls(c, qt, kt, s_ps):
                """S^T pair for both heads of chunk c (64x128 array mode)."""
                j = kt - 4 * qt
                qoff = max(j, 0) * 128
                qsl = slice(qt * 512 + qoff, (qt + 1) * 512)
                for hp in range(2):
                    po = hp * 64
                    nc.tensor.matmul(
                        s_ps[:, hp * 512 + qoff:(hp + 1) * 512],
                        lhsT=kt_sb[po:po + 64, c, kt * 128:(kt + 1) * 128],
                        rhs=qt_sb[po:po + 64, c, qsl],
                        start=True,
                        stop=True,
                    )
                return j, qoff

            def exp_stage(j, qoff, s_ps):
                p_t = ptpool.tile([128, 1024], bf16, tag="pt")
                if j >= 0:
                    w = 512 - qoff
                    s_stage = small.tile([128, 1024], f32, tag="sst")
                    sps_v = s_ps.rearrange(
                        "p (h q) -> p h q", h=2)[:, :, qoff:]
                    sst_v = s_stage.rearrange(
                        "p (h q) -> p h q", h=2)[:, :, :w]
                    nc.vector.tensor_add(
                        out=sst_v,
                        in0=sps_v,
                        in1=mask_sb[:, :w].unsqueeze(1).to_broadcast(
                            (128, 2, w)),
                    )
                    nc.scalar.activation(
                        out=p_t.rearrange(
                            "p (h q) -> p h q", h=2)[:, :, qoff:],
                        in_=sst_v,
                        func=Exp,
                        scale=0.125,
                    )
                else:
                    nc.scalar.activation(
                        out=p_t, in_=s_ps, func=Exp, scale=0.125,
                    )
                return p_t

            def pv_matmuls(c, qt, nkt, pend, pv):
                for kt, qoff, p_t in pend:
                    for hp in range(2):
                        h = 2 * c + hp
                        nc.tensor.matmul(
                            pv[hp][:, qoff:],
                            lhsT=vaug[:, kt, h * 65:(h + 1) * 65],
                            rhs=p_t[:, hp * 512 + qoff:(hp + 1) * 512],
                            start=(kt == 0),
                            stop=(kt == nkt - 1),
                        )

            # ---- PE warm-up: keep HAM at full clock through the DMA
            # wait (PE is otherwise idle 2-12us and re-throttles).
            dummy = wpool.tile([128, 512], bf16, tag="dummy")
            nc.vector.memset(dummy, 0.0)
            for wi in range(16):
                psd = psA.tile([128, 512], f32, tag="mm", name="psd")
                nc.tensor.matmul(
                    psd, lhsT=dummy[:, 0:128], rhs=dummy,
                    start=True, stop=True,
                )
                if wi in (4, 6):
                    # gate the wq/wk loads on PE progress so they stay out
                    # of the critical startup flood (value is garbage; only
                    # the dependency matters)
                    wname = "q" if wi == 4 else "k"
                    with nc.allow_low_precision(reason="dep-only copy"):
                        nc.vector.tensor_copy(
                            out=w_sb[wname][0:1, 0:1, 0:1],
                            in_=psd[0:1, 0:1])
                    for i in range(4):
                        dma_w(w_sb[wname], wq_d if wi == 4 else wk_d, i)
            # ---- prologue ----
            v_half(0, 0)
            nc.vector.memset(x_sb[0:1, 1, 0:1, 0:1], 0.0)
            nc.sync.dma_start(out=x_sb[:, 1, :, :], in_=x_d[1, :, :, :])
            v_half(0, 1)
            for ts in range(2, TS):
                nc.vector.memset(x_sb[0:1, ts, 0:1, 0:1], 0.0)
                nc.sync.dma_start(out=x_sb[:, ts, :, :], in_=x_d[ts, :, :, :])
            nc.vector.memset(wot_sb[0:1, 0:1, 0:1], 0.0)
            for i in range(4):
                nc.sync.dma_start(out=wot_sb[:, i, :], in_=wot_d[:, i, :])
            qk_half(0, 0, "q")
            qk_half(0, 0, "k")

            # ---- main loop over chunks ----
            for c in range(4):
                # filler units (each ~8-16 matmuls); deadline = q-tile index
                # in THIS chunk's iteration order before which it must run.
                fillers = []
                if c < 3:
                    for ts in range(1, TS):
                        if c == 0:
                            fillers.append((ts, lambda ts=ts: v_half(ts, 0)))
                            fillers.append((ts, lambda ts=ts: v_half(ts, 1)))
                        fillers.append(
                            (ts, lambda c=c, ts=ts: qk_half(c, ts, "q")))
                        fillers.append(
                            (ts, lambda c=c, ts=ts: qk_half(c, ts, "k")))
                    fillers.append(
                        (None, lambda cn=c + 1: qk_half(cn, 0, "q")))
                    fillers.append(
                        (None, lambda cn=c + 1: qk_half(cn, 0, "k")))
                if c == 3:
                    for ts in range(1, TS):
                        fillers.append(
                            (ts, lambda ts=ts: qk_half(3, ts, "q")))
                        fillers.append(
                            (ts, lambda ts=ts: qk_half(3, ts, "k")))

                qt_order = list(range(TS))
                total_iters = sum(2 * (qt + 1) for qt in qt_order)
                ready_fill = list(fillers)
                spacing = 2 if c == 3 else max(
                    1, total_iters // max(1, len(ready_fill) + 1))
                it = 0
                for qti, qt in enumerate(qt_order):
                    while ready_fill and ready_fill[0][0] is not None \
                            and ready_fill[0][0] <= qt:
                        ready_fill.pop(0)[1]()
                    pv = [
                        psPV.tile([65, 512], f32, tag="pv", name=f"pv{i}")
                        for i in range(2)
                    ]
                    nkt = 4 * (qt + 1)
                    pend = []            # exp'd tiles whose PV is deferred
                    for ktp in range(nkt // 2):
                        spair = []
                        for kt in (2 * ktp, 2 * ktp + 1):
                            s_ps = psS.tile([128, 1024], f32, tag="s")
                            j, qoff = s_matmuls(c, qt, kt, s_ps)
                            spair.append((kt, j, qoff, s_ps))
                        if pend:
                            pv_matmuls(c, qt, nkt, pend, pv)
                            pend = []
                        for kt, j, qoff, s_ps in spair:
                            p_t = exp_stage(j, qoff, s_ps)
                            pend.append((kt, qoff, p_t))
                        it += 1
                        if ready_fill and it % spacing == 0 and \
                                not (c == 0 and qt == 0) and \
                                not (c == 3 and len(ready_fill) <= 4):
                            ready_fill.pop(0)[1]()
                    pv_matmuls(c, qt, nkt, pend, pv)
                    if ready_fill:
                        ready_fill.pop(0)[1]()
                    stage_epilogue(c, qt, pv)
                    if c == 3:
                        for tt in range(qt * 4, qt * 4 + 4):
                            for of in range(2):
                                ready_fill.append(
                                    (None, lambda tt=tt, of=of:
                                     outproj_group(tt, of)))
                # chunk tail: remaining fillers
                for _, f in ready_fill:
                    f()

    if split_waits:
        _split_excess_waits(nc)
    return nc


# ---------------------------------------------------------------------------
# Host side
# ---------------------------------------------------------------------------

_NC_CACHE = {}


def _get_nc(t=T):
    if t not in _NC_CACHE:
        _NC_CACHE[t] = build_nc(t)
    return _NC_CACHE[t]


_MASK_CACHE = None


def make_mask():
    global _MASK_CACHE
    if _MASK_CACHE is not None:
        return _MASK_CACHE
    k = np.arange(128)[:, None]
    q = np.arange(128)[None, :]
    tri = np.where(k <= q, 0.0, MASK_NEG).astype(np.float32)
    _MASK_CACHE = np.concatenate(
        [tri, np.zeros((128, 384), np.float32)], axis=1)
    return _MASK_CACHE


def _dev_layout(a, nchunks):
    """[C_sub, F] -> [128, nchunks, F] with feature = chunk*128 + partition,
    C-contiguous so each partition row is one contiguous DMA run."""
    csub, f = a.shape
    assert csub == nchunks * 128
    return np.ascontiguousarray(
        a.reshape(nchunks, 128, f).transpose(1, 0, 2))


def core_inputs(x, Wq, bq, Wk, bk, Wv, bv, Wo, core):
    b, g = divmod(core, 2)
    gs = slice(g * G, (g + 1) * G)
    xt = np.ascontiguousarray(
        x[b].T.astype(BF16).reshape(8, 128, T // 512, 512)
        .transpose(2, 1, 0, 3))                       # [TS, 128, 8, 512]
    wqt = _dev_layout(Wq[gs, :].T.astype(BF16), 8)    # [128, 8, G]
    wkt = _dev_layout(Wk[gs, :].T.astype(BF16), 8)
    wvt = _dev_layout(Wv[gs, :].T.astype(BF16), 8)
    wot = _dev_layout(Wo[:, gs].T.astype(BF16), 4)    # [128, 4, C]
    bqk = np.concatenate(
        [bq[gs].reshape(4, 128).T, bk[gs].reshape(4, 128).T], axis=1
    ).astype(np.float32)                              # [128, 8]
    return {
        "xt": xt, "wqt": wqt, "wkt": wkt, "wvt": wvt, "wot": wot,
        "bqk": bqk, "bv": bv[gs].astype(np.float32), "mask": make_mask(),
    }


def kernel(x, Wq, bq, Wk, bk, Wv, bv, Wo, bo, _trace=False):
    x = np.asarray(x, dtype=np.float32)
    nc = _get_nc(T)
    in_maps = [
        core_inputs(x, Wq, bq, Wk, bk, Wv, bv, Wo, c) for c in range(N_CORES)
    ]
    res = run_bass_kernel_spmd(nc, in_maps, list(range(N_CORES)), trace=_trace)
    out = np.empty((B, T, C), dtype=np.float32)
    bo = np.asarray(bo, dtype=np.float32)
    for b in range(B):
        out[b] = res.results[2 * b]["out"] + res.results[2 * b + 1]["out"]
        out[b] += bo[None, :]
    kernel.last_results = res
    return out
